# revision 10
# baseline (speedup 1.0000x reference)
"""Trainium2 Bass kernel for nn_Attention_10771777978404 (sparse_attention).

Head-parallel (tensor parallel) sharding over 8 NeuronCores:
  - each core owns NH/8 = 2 heads: computes its q/k/v projections (columns of
    wq/wk/wv), RoPE, causal attention with the low-rank sigmoid gate, and a
    full-size bf16 partial of the output projection from its own heads.
  - the rank-32 adapter (gate) weights are replicated; each core stages the
    full causal sigmoid-gate tile set through DRAM once per batch.
  - host sums the 8 bf16 partials in fp32 (no device collective).

Schedule highlights (v4):
  - attention is software-pipelined: scores/exp for k-tile kt+1 are emitted
    before rowsum/AV of kt, so the ACT engine streams exps back-to-back
    instead of waiting on the exp->gate->AV->score FIFO chain.
  - each query block's normalize + output projection is deferred and spread
    into the NEXT block's k-loop (and across phase boundaries), removing the
    per-block epilogue stall.
  - front phase interleaves b0 gates with q/k pieces and b1 stream-adapters/
    gates/v/rope in rounds; both batches' sigmoids finish by ~95us so the
    single ACT Sigmoid->Exp table switch never blocks attention.
  - causal mask is a -1e9 PE accumulate (identity stationary) into the score
    PSUM; rowsum reciprocal broadcast via GpSimd partition_broadcast.
  - PSUM: score depth 3 + outproj 1 + AV 2 + rowsum 2 = 8 banks.

self-contained: hardcodes the problem shapes; only needs `concourse` (on
PYTHONPATH in this container) + jax axon devices.
"""

import math
from dataclasses import dataclass

import numpy as np
import ml_dtypes

import concourse.bass as bass
import concourse.tile as tile
from concourse import bacc
from concourse import mybir
from concourse import bass_utils
from concourse.tile_rust import add_dep_helper

BF16 = mybir.dt.bfloat16
F32 = mybir.dt.float32
AF = mybir.ActivationFunctionType


@dataclass(frozen=True)
class Cfg:
    B: int = 2
    S: int = 2048
    DIM: int = 2048
    NH: int = 16
    HD: int = 128
    RANK: int = 32
    NCORES: int = 8
    QT: int = 512   # query block (free dim of score tiles)
    KT: int = 128   # key block (partition dim of score tiles)

    @property
    def HLOC(self):
        return self.NH // self.NCORES

    @property
    def DH(self):
        return self.HLOC * self.HD  # per-core head-dim span

    @property
    def KTILES(self):
        return self.DIM // 128  # contraction tiles for projections

    @property
    def QTN(self):
        return self.S // self.QT

    @property
    def DIAG(self):
        return self.QT // self.KT  # k-tiles per diagonal band

    @property
    def TBLK(self):
        return self.DIAG * self.QTN * (self.QTN + 1) // 2


FULL = Cfg()


def build_nc(cfg: Cfg = FULL, *, use_gate=True, use_rope=True, use_mask=True):
    c = cfg
    assert c.HD == 128 and c.KT == 128
    nc = bacc.Bacc("TRN2", target_bir_lowering=False, debug=False,
                   num_devices=c.NCORES)

    # ---- kernel I/O ----
    xT = nc.dram_tensor("xT", [c.B, c.DIM, c.S], BF16, kind="ExternalInput")
    wqT = nc.dram_tensor("wqT", [c.DIM, c.DH], BF16, kind="ExternalInput")
    wkT = nc.dram_tensor("wkT", [c.DIM, c.DH], BF16, kind="ExternalInput")
    wvT = nc.dram_tensor("wvT", [c.DIM, c.DH], BF16, kind="ExternalInput")
    wocT = nc.dram_tensor("wocT", [c.DH, c.DIM], BF16, kind="ExternalInput")
    waT = nc.dram_tensor("waT", [c.DIM, 2 * c.RANK], BF16, kind="ExternalInput")
    c2d = nc.dram_tensor("c2d", [c.HD, c.S], BF16, kind="ExternalInput")
    s2d = nc.dram_tensor("s2d", [c.HD, c.S], BF16, kind="ExternalInput")
    pswapd = nc.dram_tensor("pswapd", [c.HD, c.HD], BF16, kind="ExternalInput")
    identd = nc.dram_tensor("identd", [c.KT, c.KT], BF16, kind="ExternalInput")
    # additive causal mask bands (0 / -1e9), [j][k, q]
    maskdd = nc.dram_tensor("maskdd", [c.DIAG, c.KT, c.QT], BF16, kind="ExternalInput")

    # partial output projection, transposed, bf16: pout[j, b*S + t]
    pout = nc.dram_tensor("pout", [c.DIM, c.B * c.S], BF16, kind="ExternalOutput")

    # gate tiles sigmoid(A')[k, q] staged via DRAM (Exp and Sigmoid live in
    # different ACT tables; all sigmoids for BOTH batches run before any exp)
    gdram = nc.dram_tensor("gdram", [c.B, c.TBLK, c.KT, c.QT], BF16)

    isqrt = 1.0 / math.sqrt(c.HD)
    NQC = c.DH // 128          # per-core q/k head chunks (= HLOC)

    from contextlib import ExitStack
    with ExitStack() as _ctx:
        tc = _ctx.enter_context(tile.TileContext(nc))
        cst = _ctx.enter_context(tc.tile_pool(name="const", bufs=1))
        xtp = _ctx.enter_context(tc.tile_pool(name="xt", bufs=1))
        xsp = _ctx.enter_context(tc.tile_pool(name="xs", bufs=3))
        adp = _ctx.enter_context(tc.tile_pool(name="ap", bufs=1))
        qkp = _ctx.enter_context(tc.tile_pool(name="qk", bufs=1))
        vp = _ctx.enter_context(tc.tile_pool(name="vp", bufs=1))
        rtp = _ctx.enter_context(tc.tile_pool(name="rope_t", bufs=2))
        gio = _ctx.enter_context(tc.tile_pool(name="gio", bufs=4))
        pge = _ctx.enter_context(tc.tile_pool(name="pge", bufs=3))
        nrm = _ctx.enter_context(tc.tile_pool(name="norm", bufs=2))
        f2p = _ctx.enter_context(tc.tile_pool(name="f2", bufs=2))
        # PSUM banks (8): tag "ps" = gates + attention scores (2); tag
        # "pj" = projections + outproj (2); po = AV accumulators (2);
        # prs0/prs1 = per-head rowsum accumulators (1+1)
        pp = _ctx.enter_context(tc.tile_pool(name="pp", bufs=2, space="PSUM"))
        pop = _ctx.enter_context(tc.tile_pool(name="po", bufs=2, space="PSUM"))
        prsp = _ctx.enter_context(tc.tile_pool(name="prs", bufs=1, space="PSUM"))

        # ---- priming DMAs: wa + x(b0) first so adapters start immediately --
        wa_sb = cst.tile([128, c.KTILES, 2 * c.RANK], BF16, name="wa_sb")
        nc.sync.dma_start(out=wa_sb, in_=waT.ap().rearrange("(t p) m -> p t m", p=128))

        xt_tiles = {}

        def load_xt(b, engine):
            xt_sb = xtp.tile([128, c.KTILES, c.S], BF16, name="xt_sb", tag="xt")
            xr = xT.ap()[b].rearrange("(t p) n -> p t n", p=128)
            for kt in range(c.KTILES):
                engine.dma_start(out=xt_sb[:, kt, :], in_=xr[:, kt, :])
            xt_tiles[b] = xt_sb

        load_xt(0, nc.sync)

        wq_sb = cst.tile([128, c.KTILES, c.DH], BF16, name="wq_sb")
        wk_sb = cst.tile([128, c.KTILES, c.DH], BF16, name="wk_sb")
        wv_sb = cst.tile([128, c.KTILES, c.DH], BF16, name="wv_sb")
        woc_sb = cst.tile([128, NQC, c.DIM], BF16, name="woc_sb")
        c2_sb = cst.tile([128, c.S], BF16, name="c2_sb")
        s2_sb = cst.tile([128, c.S], BF16, name="s2_sb")
        psw_sb = cst.tile([128, 128], BF16, name="psw_sb")
        id_sb = cst.tile([128, 128], BF16, name="id_sb")
        mask_sb = cst.tile([128, c.DIAG, c.QT], BF16, name="mask_sb")
        ones_sb = cst.tile([128, 1], BF16, name="ones_sb")

        for w_sb, w_d in ((wq_sb, wqT), (wk_sb, wkT)):
            wr = w_d.ap().rearrange("(t p) m -> p t m", p=128)
            for half in range(2):
                h0 = half * (c.KTILES // 2)
                nc.sync.dma_start(out=w_sb[:, h0:h0 + c.KTILES // 2, :],
                                  in_=wr[:, h0:h0 + c.KTILES // 2, :])
        nc.sync.dma_start(out=c2_sb, in_=c2d.ap())
        nc.sync.dma_start(out=s2_sb, in_=s2d.ap())
        nc.sync.dma_start(out=psw_sb, in_=pswapd.ap())
        nc.sync.dma_start(out=id_sb, in_=identd.ap())
        nc.sync.dma_start(out=mask_sb, in_=maskdd.ap().rearrange("j p q -> p j q"))
        wr = wvT.ap().rearrange("(t p) m -> p t m", p=128)
        for half in range(2):
            h0 = half * (c.KTILES // 2)
            nc.sync.dma_start(out=wv_sb[:, h0:h0 + c.KTILES // 2, :],
                              in_=wr[:, h0:h0 + c.KTILES // 2, :])
        wcr = wocT.ap().rearrange("(h p) j -> p h j", p=128)
        for h in range(NQC):
            nc.sync.dma_start(out=woc_sb[:, h, :], in_=wcr[:, h, :])
        nc.vector.memset(ones_sb, 1.0)

        # ---- adapters: fused aq|ak projection -> a2t rows [aq(0:32); ak(32:64)]
        a2 = {}   # b -> (a2t with aq in rows 0:32, ak realigned tile)

        def emit_adapters_resident(b):
            a2t = adp.tile([64, c.S], BF16, name="a2t", tag="a2t")
            xt_sb = xt_tiles[b]
            for qt in range(c.QTN):
                pa = pp.tile([64, c.QT], F32, name="pa", tag="pj", bufs=2)
                for kt in range(c.KTILES):
                    nc.tensor.matmul(pa[:, :], wa_sb[:, kt, :],
                                     xt_sb[:, kt, qt * c.QT:(qt + 1) * c.QT],
                                     start=(kt == 0), stop=(kt == c.KTILES - 1))
                nc.vector.tensor_copy(a2t[:, qt * c.QT:(qt + 1) * c.QT], pa[:, :])
            ak_sb = adp.tile([32, c.S], BF16, name="ak_sb", tag="ak")
            # realign ak rows to base partition 0 (SBUF->SBUF partition shift)
            nc.sync.dma_start(out=ak_sb[:, :], in_=a2t[32:64, :])
            a2[b] = (a2t, ak_sb)

        def emit_adapters_stream_qt(b, qt, a2t, ak_sb):
            # one query block of batch-b adapters from streamed x chunks,
            # with its ak rows realigned immediately (per-qt) so gates for
            # this block can start without waiting for the whole batch
            xr = xT.ap()[b].rearrange("(t p) n -> p t n", p=128)
            qsl = slice(qt * c.QT, (qt + 1) * c.QT)
            pa = pp.tile([64, c.QT], F32, name="pa", tag="pj", bufs=2)
            for ktp in range(c.KTILES // 2):
                xs = xsp.tile([128, 2, c.QT], BF16, name="xs", tag="xs")
                nc.gpsimd.dma_start(out=xs, in_=xr[:, 2 * ktp:2 * ktp + 2, qsl])
                for j in range(2):
                    kt = 2 * ktp + j
                    nc.tensor.matmul(pa[:, :], wa_sb[:, kt, :], xs[:, j, :],
                                     start=(kt == 0), stop=(kt == c.KTILES - 1))
            nc.vector.tensor_copy(a2t[:, qsl], pa[:, :])
            nc.sync.dma_start(out=ak_sb[:, qsl], in_=a2t[32:64, qsl])

        last_sig = [None]

        def emit_gates_qt(b, qt):
            a2t, ak_sb = a2[b]
            qsl = slice(qt * c.QT, (qt + 1) * c.QT)
            for kt in range(c.DIAG * (qt + 1)):
                ksl = slice(kt * c.KT, (kt + 1) * c.KT)
                off = (qt * (qt + 1) // 2) * c.DIAG + kt
                pg = pp.tile([128, c.QT], F32, name="pg", tag="ps")
                nc.tensor.matmul(pg[:, :], ak_sb[:, ksl], a2t[0:32, qsl],
                                 start=True, stop=True)
                gout = gio.tile([128, c.QT], BF16, name="gout", tag="gout")
                sig = nc.scalar.activation(gout[:, :], pg[:, :], AF.Sigmoid)
                last_sig[0] = sig.ins
                nc.sync.dma_start(out=gdram.ap()[b, off], in_=gout[:, :])

        qk_tiles = {}

        def make_qk_tiles(b):
            qk_tiles[b] = {
                "q": [qkp.tile([128, c.S], BF16, name=f"q{h}_sb", tag=f"q{h}")
                      for h in range(NQC)],
                "k": [qkp.tile([128, c.S], BF16, name=f"k{h}_sb", tag=f"k{h}")
                      for h in range(NQC)],
            }

        def emit_qk_piece(b, which, h):
            xt_sb = xt_tiles[b]
            w = wq_sb if which == "q" else wk_sb
            dst = qk_tiles[b][which][h]
            for qt in range(c.QTN):
                psum = pp.tile([128, c.QT], F32, name="psum_qk", tag="pj", bufs=2)
                for kt in range(c.KTILES):
                    nc.tensor.matmul(
                        psum[:, :],
                        w[:, kt, h * 128:(h + 1) * 128],
                        xt_sb[:, kt, qt * c.QT:(qt + 1) * c.QT],
                        start=(kt == 0), stop=(kt == c.KTILES - 1))
                nc.vector.tensor_copy(dst[:, qt * c.QT:(qt + 1) * c.QT], psum[:, :])

        v_tiles = {}

        def make_v_tile(b):
            v_tiles[b] = vp.tile([128, c.S // 128, c.DH], BF16, name="v_sb", tag="v")

        def emit_v_piece(b, tts):
            xt_sb = xt_tiles[b]
            v_sb = v_tiles[b]
            for tt in tts:
                psum = pp.tile([128, c.DH], F32, name="psum_v", tag="pj", bufs=2)
                for kt in range(c.KTILES):
                    nc.tensor.matmul(
                        psum[:, :],
                        xt_sb[:, kt, tt * 128:(tt + 1) * 128],
                        wv_sb[:, kt, :],
                        start=(kt == 0), stop=(kt == c.KTILES - 1))
                nc.vector.tensor_copy(v_sb[:, tt, :], psum[:, :])

        def emit_rope_piece(b, which, h):
            if not use_rope:
                return
            t = qk_tiles[b][which][h]
            for qt in range(c.QTN):
                sl = slice(qt * c.QT, (qt + 1) * c.QT)
                pswp = pp.tile([128, c.QT], F32, name="pswp", tag="pj", bufs=2)
                nc.tensor.matmul(pswp[:, :], psw_sb[:, :], t[:, sl],
                                 start=True, stop=True)
                m1 = rtp.tile([128, c.QT], BF16, name="rope_m1", tag="m1")
                m2 = rtp.tile([128, c.QT], BF16, name="rope_m2", tag="m2")
                nc.vector.tensor_mul(m1[:, :], t[:, sl], c2_sb[:, sl])
                nc.vector.tensor_mul(m2[:, :], pswp[:, :], s2_sb[:, sl])
                nc.vector.tensor_add(t[:, sl], m1[:, :], m2[:, :])

        first_exp = [True]

        def emit_attention(b, carry=None):
            """Software-pipelined attention for batch b.

            Returns the last query block's epilogue closures (normalize +
            output projection) for the caller to spread into later emission.
            `carry` is such a list from earlier context.
            """
            q_sb = qk_tiles[b]["q"]
            k_sb = qk_tiles[b]["k"]
            v_sb = v_tiles[b]
            spread = list(carry or [])

            def drain(n):
                for _ in range(min(n, len(spread))):
                    spread.pop(0)()

            def make_epilogue(qt, po, prs2):
                qsl = slice(qt * c.QT, (qt + 1) * c.QT)
                og = nrm.tile([128, c.HLOC, c.QT], BF16, name="og", tag="og")

                def pre():
                    for h in range(c.HLOC):
                        rr = nrm.tile([1, c.QT], F32, name="rr", tag="rr")
                        nc.vector.reciprocal_approx_fast(out=rr[:, :],
                                                         in_=prs2[h][:, :])
                        rbc = nrm.tile([128, c.QT], F32, name="rbc", tag="rbc")
                        nc.gpsimd.partition_broadcast(rbc[:, :], rr[:, :])
                        nc.vector.tensor_mul(og[:, h, :], po[h][:, :], rbc[:, :])

                NCH = c.DIM // 128
                units = [pre]
                f2_holder = {}

                def make_unit(u):
                    def unit():
                        half, jj = divmod(u, NCH // 2)
                        if jj == 0:
                            f2_holder[half] = f2p.tile(
                                [128, NCH // 2, c.QT], BF16, name="f2", tag="f2")
                        f2 = f2_holder[half]
                        ch = u
                        pf = pp.tile([128, c.QT], F32, name="pf", tag="pj", bufs=2)
                        for h in range(c.HLOC):
                            nc.tensor.matmul(
                                pf[:, :],
                                woc_sb[:, h, ch * 128:(ch + 1) * 128],
                                og[:, h, :],
                                start=(h == 0), stop=(h == c.HLOC - 1))
                        if u % 3 == 2:
                            nc.scalar.copy(f2[:, jj, :], pf[:, :])
                        else:
                            nc.vector.tensor_copy(f2[:, jj, :], pf[:, :])
                        if jj == NCH // 2 - 1:
                            nc.sync.dma_start(
                                out=pout.ap().rearrange("(c p) t -> p c t", p=128)[
                                    :, half * (NCH // 2):(half + 1) * (NCH // 2),
                                    b * c.S + qt * c.QT: b * c.S + (qt + 1) * c.QT],
                                in_=f2[:, :, :])
                    return unit

                units.extend(make_unit(u) for u in range(NCH))
                return units

            for qt in range(c.QTN):
                qsl = slice(qt * c.QT, (qt + 1) * c.QT)
                nkt = c.DIAG * (qt + 1)  # causal k tiles
                po = [pop.tile([128, c.QT], F32, name=f"po{h}", tag="po")
                      for h in range(c.HLOC)]
                prs2 = [prsp.tile([1, c.QT], F32, name=f"prs{h}", tag=f"prs{h}")
                        for h in range(c.HLOC)]
                pend = None  # deferred rowsum+AV emission for the previous kt
                for kt in range(nkt):
                    ksl = slice(kt * c.KT, (kt + 1) * c.KT)
                    off = (qt * (qt + 1) // 2) * c.DIAG + kt
                    gin = gio.tile([128, c.QT], BF16, name="gin", tag="gin")
                    if use_gate:
                        nc.sync.dma_start(out=gin[:, :], in_=gdram.ap()[b, off])
                    else:
                        nc.vector.memset(gin[:, :], 1.0)
                    j = kt - c.DIAG * qt
                    cur = []
                    for h in range(c.HLOC):
                        ps = pp.tile([128, c.QT], F32, name="ps", tag="ps")
                        diag = (j >= 0 and use_mask)
                        nc.tensor.matmul(ps[:, :], k_sb[h][:, ksl],
                                         q_sb[h][:, qsl],
                                         start=True, stop=not diag)
                        if diag:
                            # causal mask: accumulate -1e9 band into the score
                            # PSUM via identity-stationary matmul (PE, not DVE)
                            nc.tensor.matmul(ps[:, :], id_sb[:, :],
                                             mask_sb[:, j, :],
                                             start=False, stop=True)
                        p_sb = pge.tile([128, c.QT], BF16, name="p_sb", tag="p", bufs=5)
                        ex = nc.scalar.activation(p_sb[:, :], ps[:, :], AF.Exp,
                                                  scale=isqrt)
                        if first_exp[0] and use_gate and last_sig[0] is not None:
                            add_dep_helper(ex.ins, last_sig[0],
                                           reason="ACT table: exps after all sigmoids")
                            first_exp[0] = False
                        pgm = pge.tile([128, c.QT], BF16, name="pgm", tag="pgm", bufs=4)
                        nc.vector.tensor_mul(pgm[:, :], p_sb[:, :], gin[:, :])
                        cur.append((h, p_sb, pgm))
                    if kt == 0:
                        # previous query block's normalize runs while this
                        # block's first exps stream
                        drain(1)

                    def rsav(items, kt=kt):
                        for h, p_sb, pgm in items:
                            nc.tensor.matmul(prs2[h][:, :], ones_sb[:, :],
                                             p_sb[:, :],
                                             start=(kt == 0), stop=(kt == nkt - 1))
                            nc.tensor.matmul(po[h][:, :],
                                             v_sb[:, kt, h * 128:(h + 1) * 128],
                                             pgm[:, :],
                                             start=(kt == 0), stop=(kt == nkt - 1))
                    if pend is not None:
                        pend()
                        drain(2)
                    pend = lambda items=cur, kt=kt: rsav(items, kt)
                pend()
                spread.extend(make_epilogue(qt, po, prs2))
            return spread

        # ================= schedule =================
        emit_adapters_resident(0)
        make_qk_tiles(0)
        # gates(b0, qt) interleaved with q/k pieces: PE stays dense while the
        # ACT engine chews sigmoids
        qk_pieces = [("q", 0), ("q", 1), ("k", 0), ("k", 1)]
        for r in range(c.QTN):
            if use_gate:
                emit_gates_qt(0, r)
            emit_qk_piece(0, *qk_pieces[r])
        # rounds: b1 stream-adapters + b1 gates + b0 v + b0 rope
        make_v_tile(0)
        if use_gate:
            a2t_1 = adp.tile([64, c.S], BF16, name="a2t", tag="a2t")
            ak_1 = adp.tile([32, c.S], BF16, name="ak_sb", tag="ak")
            a2[1] = (a2t_1, ak_1)
        for r in range(c.QTN):
            if use_gate:
                emit_adapters_stream_qt(1, r, a2t_1, ak_1)
                emit_gates_qt(1, r)
            emit_v_piece(0, range(4 * r, 4 * r + 4))
            emit_rope_piece(0, *qk_pieces[r])
        # b1 x reload from the GpSimd queue (overlaps b0 attention; never
        # head-of-line blocks the Sync queue's gate-tile reads)
        load_xt(1, nc.gpsimd)
        make_qk_tiles(1)
        make_v_tile(1)

        carry = emit_attention(0)
        # b1 projections, with b0's last-block epilogue spread between pieces
        for piece in qk_pieces:
            emit_qk_piece(1, *piece)
            for _ in range(4):
                if carry:
                    carry.pop(0)()
        while carry:
            carry.pop(0)()
        emit_v_piece(1, range(c.S // 128))
        for piece in qk_pieces:
            emit_rope_piece(1, *piece)
        tail = emit_attention(1)
        while tail:
            tail.pop(0)()

    nc.compile()
    return nc


def make_core_inputs(inputs: dict, cfg: Cfg = FULL):
    """Host-side sharding: returns in_maps (one dict per core)."""
    c = cfg
    bf16 = ml_dtypes.bfloat16
    x = np.asarray(inputs["x"])
    mask = np.asarray(inputs["mask"])
    fc = np.asarray(inputs["freqs_cos"])
    fs = np.asarray(inputs["freqs_sin"])
    wq, wk, wv, wo = (np.asarray(inputs[k]) for k in ("wq", "wk", "wv", "wo"))
    wa_q, wa_k = np.asarray(inputs["wa_q"]), np.asarray(inputs["wa_k"])

    xT = np.ascontiguousarray(x.transpose(0, 2, 1)).astype(bf16)
    waT = np.ascontiguousarray(np.concatenate([wa_q, wa_k], axis=0).T).astype(bf16)

    # rope tables in [d, tok] layout
    c2 = np.empty((c.HD, c.S), np.float32)
    s2 = np.empty((c.HD, c.S), np.float32)
    c2[0::2] = fc.T
    c2[1::2] = fc.T
    s2[0::2] = -fs.T
    s2[1::2] = fs.T
    c2 = c2.astype(bf16)
    s2 = s2.astype(bf16)

    psw = np.zeros((c.HD, c.HD), np.float32)
    idx = np.arange(c.HD)
    psw[idx, idx ^ 1] = 1.0
    psw = psw.astype(bf16)
    ident = np.eye(c.KT, dtype=np.float32).astype(bf16)

    # additive diagonal-band mask patterns [j][k, q] (0 / -1e9)
    qt_last = c.QTN - 1
    q0 = qt_last * c.QT
    maskd = np.empty((c.DIAG, c.KT, c.QT), np.float32)
    for j in range(c.DIAG):
        k0 = (c.DIAG * qt_last + j) * c.KT
        maskd[j] = mask[0, 0, q0:q0 + c.QT, k0:k0 + c.KT].T
    maskd = maskd.astype(bf16)

    in_maps = []
    for ci in range(c.NCORES):
        rows = slice(ci * c.DH, (ci + 1) * c.DH)
        in_maps.append({
            "xT": xT,
            "wqT": np.ascontiguousarray(wq[rows].T).astype(bf16),
            "wkT": np.ascontiguousarray(wk[rows].T).astype(bf16),
            "wvT": np.ascontiguousarray(wv[rows].T).astype(bf16),
            "wocT": np.ascontiguousarray(wo[:, rows].T).astype(bf16),
            "waT": waT,
            "c2d": c2,
            "s2d": s2,
            "pswapd": psw,
            "identd": ident,
            "maskdd": maskd,
        })
    return in_maps


def assemble_output(results, cfg: Cfg = FULL) -> np.ndarray:
    c = cfg
    total = np.zeros((c.DIM, c.B * c.S), np.float32)
    for ci in range(c.NCORES):
        total += np.asarray(results[ci]["pout"]).astype(np.float32)
    return np.ascontiguousarray(
        total.reshape(c.DIM, c.B, c.S).transpose(1, 2, 0))


_NC_CACHE = {}


def run(nc, in_maps, trace=False, cfg: Cfg = FULL, **kw):
    return bass_utils.run_bass_kernel_spmd(
        nc, in_maps, core_ids=list(range(cfg.NCORES)), trace=trace, **kw)


def kernel(**inputs) -> np.ndarray:
    cfg = FULL
    if cfg not in _NC_CACHE:
        _NC_CACHE[cfg] = build_nc(cfg)
    nc = _NC_CACHE[cfg]
    in_maps = make_core_inputs(inputs, cfg)
    res = run(nc, in_maps, cfg=cfg)
    return assemble_output(res.results, cfg)


if __name__ == "__main__":
    nc = build_nc(FULL)
    print("built ok")
